# revision 5
# baseline (speedup 1.0000x reference)
"""Multi-headed attention (B=2, S=2048, H=12, D=64, hidden=768) on 8 NeuronCores.

Sharding: 8 cores = 2 batches x 4 head-groups (3 heads each). Per core:
  - hT and all weights arrive in bf16 (halves input DMA); Q/K projections
    use duplicated weight columns so psum holds [Q;Q] / [K;K] across the
    two partition halves.
  - K is split into a bf16 hi/lo pair stacked in partitions 0-63 / 64-127
    (K at ~16-bit mantissa of the bf16-projected value).
  - scores^T per k-tile = one K=128 bf16 matmul: lhsT=[Khi;Klo], rhs=[Q;Q].
  - E = exp(0.125*s + mask[k]) on ACT (mask is the per-partition bias), bf16.
  - ctx = E^T-accumulated @ [V | 1] in psum: unnormalized ctx rides in cols
    0-63, the softmax denominator in col 64 (ones column of augmented V);
    epilogue = per-partition reciprocal * mul, then per-head output DMA.
"""

import numpy as np
import ml_dtypes

import concourse.bass as bass
import concourse.mybir as mybir
import concourse.tile as tile
from concourse import bacc
from concourse.bass_utils import run_bass_kernel_spmd

F = 768          # hidden
D = 64           # head dim
HPC = 3          # heads per core
FC = F // 128    # contraction chunks

_cache = {}


def _build(S):
    NT = S // 128           # token tiles
    QC = S // 512           # 512-wide q chunks
    f32 = mybir.dt.float32
    bf16 = mybir.dt.bfloat16
    EXP = mybir.ActivationFunctionType.Exp

    nc = bacc.Bacc("TRN2", target_bir_lowering=False, debug=False, num_devices=8)
    hT = nc.dram_tensor("hT", [F, S], bf16, kind="ExternalInput").ap()
    wqd = nc.dram_tensor("wqd", [F, HPC * 128], bf16, kind="ExternalInput").ap()
    wkd = nc.dram_tensor("wkd", [F, HPC * 128], bf16, kind="ExternalInput").ap()
    wv = nc.dram_tensor("wv", [F, HPC * D], bf16, kind="ExternalInput").ap()
    mask = nc.dram_tensor("mask", [S], f32, kind="ExternalInput").ap()
    out = nc.dram_tensor("out", [S, HPC * D], f32, kind="ExternalOutput").ap()

    with tile.TileContext(nc) as tc:
        with (
            tc.tile_pool(name="const", bufs=1) as cpool,
            tc.tile_pool(name="epool", bufs=4) as epool,
            tc.tile_pool(name="tpool", bufs=2) as tpool,
            tc.tile_pool(name="rcpool", bufs=3) as rcpool,
            tc.tile_pool(name="ps_small", bufs=4, space="PSUM") as pps,
            tc.tile_pool(name="ps_sc", bufs=2, space="PSUM") as ppsc,
        ):
            hT_sb = cpool.tile([128, FC * S], bf16, tag="hT")
            wqd_sb = cpool.tile([128, FC * HPC * 128], bf16, tag="wqd")
            wkd_sb = cpool.tile([128, FC * HPC * 128], bf16, tag="wkd")
            wv_sb = cpool.tile([128, FC * HPC * D], bf16, tag="wv")
            mask_sb = cpool.tile([128, NT], f32, tag="mask")
            qd = cpool.tile([128, HPC * S], bf16, tag="qd")    # [Q;Q] per head
            khl = cpool.tile([128, HPC * S], bf16, tag="khl")  # [Khi;Klo] per head
            vsb = cpool.tile([128, NT * 195], bf16, tag="vsb")
            out_sb = cpool.tile([128, NT * HPC * D], f32, tag="out")

            # Batched DMAs: each dma_start fans out over all 16 SDMA engines;
            # spread issue over the SP / ACT / Pool rings so they don't
            # serialize behind one FIFO.
            nc.gpsimd.dma_start(
                out=mask_sb[:, :], in_=mask.rearrange("(c p) -> p c", p=128)
            )
            nc.sync.dma_start(
                out=wqd_sb.rearrange("p (fc c) -> p fc c", fc=FC),
                in_=wqd.rearrange("(fc p) c -> p fc c", p=128),
            )
            nc.scalar.dma_start(
                out=wkd_sb.rearrange("p (fc c) -> p fc c", fc=FC),
                in_=wkd.rearrange("(fc p) c -> p fc c", p=128),
            )
            nc.gpsimd.dma_start(
                out=wv_sb.rearrange("p (fc c) -> p fc c", fc=FC),
                in_=wv.rearrange("(fc p) c -> p fc c", p=128),
            )
            hT_r = hT.rearrange("(fc p) s -> p fc s", p=128)
            hT_sbr = hT_sb.rearrange("p (fc s) -> p fc s", s=S)
            for qc in range(QC):
                c0, c1 = qc * 512, (qc + 1) * 512
                nc.sync.dma_start(
                    out=hT_sbr[:, :, c0:c1],
                    in_=hT_r[:, :, c0:c1],
                )
            nc.vector.memset(
                vsb.rearrange("p (t c) -> p t c", c=65)[:, :, 64:65], 1.0
            )

            def q_pass(h, qc):
                """qd[h] chunk: bf16 matmuls with duplicated Wq -> [Q;Q]."""
                ps = pps.tile([128, 512], f32, tag="ps1", name=f"psq_{h}_{qc}")
                for fc in range(FC):
                    nc.tensor.matmul(
                        ps[:, :],
                        wqd_sb[:, fc * HPC * 128 + h * 128: fc * HPC * 128 + (h + 1) * 128],
                        hT_sb[:, fc * S + qc * 512: fc * S + (qc + 1) * 512],
                        start=(fc == 0), stop=(fc == FC - 1),
                    )
                nc.vector.tensor_copy(
                    out=qd[:, h * S + qc * 512: h * S + (qc + 1) * 512],
                    in_=ps[:, :],
                )

            def k_pass(h, qc):
                """khl[h] chunk: bf16 matmuls (dup Wk) -> bf16 hi/lo split."""
                ps = pps.tile([128, 512], f32, tag="ps1", name=f"psk_{h}_{qc}")
                for fc in range(FC):
                    nc.tensor.matmul(
                        ps[:, :],
                        wkd_sb[:, fc * HPC * 128 + h * 128: fc * HPC * 128 + (h + 1) * 128],
                        hT_sb[:, fc * S + qc * 512: fc * S + (qc + 1) * 512],
                        start=(fc == 0), stop=(fc == FC - 1),
                    )
                tmp = tpool.tile([128, 512], bf16, tag="ktmp", name=f"ktmp_{h}_{qc}")
                nc.vector.tensor_copy(out=tmp[:, :], in_=ps[:, :])
                sl = slice(h * S + qc * 512, h * S + (qc + 1) * 512)
                nc.vector.tensor_copy(out=khl[0:64, sl], in_=tmp[0:64, :])
                nc.vector.tensor_sub(khl[64:128, sl], ps[64:128, :], tmp[64:128, :])

            def v_tile(tt):
                ps = pps.tile([128, 512], f32, tag="ps1", name=f"psv_{tt}")
                for fc in range(FC):
                    nc.tensor.matmul(
                        ps[:, 0:HPC * D],
                        hT_sb[:, fc * S + tt * 128: fc * S + tt * 128 + 128],
                        wv_sb[:, fc * HPC * D:(fc + 1) * HPC * D],
                        start=(fc == 0), stop=(fc == FC - 1),
                    )
                for h in range(HPC):
                    nc.vector.tensor_copy(
                        out=vsb[:, tt * 195 + h * 65: tt * 195 + h * 65 + 64],
                        in_=ps[:, h * D:(h + 1) * D],
                    )

            # Minimal prologue: just enough for the first scores of head 0.
            q_pass(0, 0)
            q_pass(0, 1)
            k_pass(0, 0)

            # Remaining projection passes, threaded through the k-loops.
            # Constraint: k_pass(h, c) must complete before head h's scores
            # for k-tiles 4c..4c+3; q chunks 2,3 of head 0 are emitted inside
            # iteration k=0 between the two eh score groups.
            work = {
                0: [("k", 0, 1), ("k", 0, 2), ("k", 0, 3),
                    ("q", 1, 0), ("q", 1, 1), ("q", 1, 2), ("q", 1, 3),
                    ("k", 1, 0), ("k", 1, 1), ("k", 1, 2), ("k", 1, 3)],
                1: [("q", 2, 0), ("q", 2, 1), ("q", 2, 2), ("q", 2, 3),
                    ("k", 2, 0), ("k", 2, 1), ("k", 2, 2), ("k", 2, 3)],
                2: [],
            }

            outr = out.rearrange("(j p) c -> p j c", p=128)
            out_sbr = out_sb.rearrange("p (j c) -> p j c", c=HPC * D)
            for h in range(HPC):
                ctx_ts = [
                    pps.tile([128, 512], f32, tag="ps1", name=f"ctx_h{h}_{i}")
                    for i in range((NT + 6) // 7)
                ]
                wq = list(work[h])
                for k in range(NT):
                    E_t = epool.tile([128, S], bf16, tag="E")
                    EW = min(1024, S)
                    for eh in range(S // EW):
                        ps = ppsc.tile([128, EW], f32, tag="ps_sc", name=f"sc_{h}_{k}_{eh}")
                        for qq in range(EW // 512):
                            q0 = eh * EW + qq * 512
                            nc.tensor.matmul(
                                ps[:, qq * 512:(qq + 1) * 512],
                                khl[:, h * S + k * 128: h * S + (k + 1) * 128],
                                qd[:, h * S + q0: h * S + q0 + 512],
                                start=True, stop=True,
                            )
                        nc.scalar.activation(
                            out=E_t[:, eh * EW:(eh + 1) * EW],
                            in_=ps[:, :],
                            func=EXP,
                            bias=mask_sb[:, k:k + 1],
                            scale=0.125,
                        )
                        if h == 0 and k == 0 and eh == 0:
                            q_pass(0, 2)
                            q_pass(0, 3)
                    # one deferred projection pass per iteration, after the
                    # scores (so EXP is never gated on projection work)
                    if wq:
                        kind, hh, qc = wq.pop(0)
                        (q_pass if kind == "q" else k_pass)(hh, qc)
                    if h == 0:
                        v_tile(k)
                    for j in range(NT):
                        ct = ctx_ts[j // 7]
                        off = (j % 7) * 66
                        nc.tensor.matmul(
                            ct[:, off:off + 65],
                            E_t[:, j * 128:(j + 1) * 128],
                            vsb[:, k * 195 + h * 65: k * 195 + (h + 1) * 65],
                            start=(k == 0 and j % 7 == 0), stop=(k == NT - 1),
                            skip_group_check=True,
                        )
                rc = rcpool.tile([128, NT], f32, tag="rc", name=f"rc_{h}")
                for j in range(NT):
                    ct = ctx_ts[j // 7]
                    off = (j % 7) * 66
                    nc.vector.reciprocal(out=rc[:, j:j + 1], in_=ct[:, off + 64:off + 65])
                    osl = out_sb[:, j * HPC * D + h * D: j * HPC * D + (h + 1) * D]
                    if h == HPC - 1 and j % 2 == 1:
                        # last head: split the scale-muls with the (now idle)
                        # ACT engine to shorten the tail
                        nc.scalar.mul(osl, ct[:, off:off + 64], rc[:, j:j + 1])
                    else:
                        nc.vector.tensor_scalar_mul(
                            osl, ct[:, off:off + 64], rc[:, j:j + 1]
                        )
                # stream this head's output while the next head computes
                JG = NT // 2
                for jg in range(0, NT, JG):
                    nc.sync.dma_start(
                        out=outr[:, jg:jg + JG, h * D:(h + 1) * D],
                        in_=out_sbr[:, jg:jg + JG, h * D:(h + 1) * D],
                    )
    nc.compile()
    return nc


def get_module(S=2048):
    if S not in _cache:
        _cache[S] = _build(S)
    return _cache[S]


def _core_inputs(hidden_states, attention_mask, Wq, Wk, Wv, c):
    bf = ml_dtypes.bfloat16
    b, g = divmod(c, 4)
    h0 = g * HPC
    wqd = np.empty((F, HPC * 128), bf)
    wkd = np.empty((F, HPC * 128), bf)
    wqb = Wq.astype(bf)
    wkb = Wk.astype(bf)
    for h in range(HPC):
        col = slice((h0 + h) * D, (h0 + h + 1) * D)
        wqd[:, h * 128:h * 128 + 64] = wqb[:, col]
        wqd[:, h * 128 + 64:(h + 1) * 128] = wqb[:, col]
        wkd[:, h * 128:h * 128 + 64] = wkb[:, col]
        wkd[:, h * 128 + 64:(h + 1) * 128] = wkb[:, col]
    return {
        "hT": np.ascontiguousarray(hidden_states[b].T.astype(bf)),
        "wqd": wqd,
        "wkd": wkd,
        "wv": np.ascontiguousarray(Wv[:, h0 * D:(h0 + HPC) * D].astype(bf)),
        "mask": np.ascontiguousarray(attention_mask[b, 0, 0, :]),
    }


def kernel(hidden_states, attention_mask, Wq, bq, Wk, bk, Wv, bv):
    hidden_states = np.asarray(hidden_states, dtype=np.float32)
    attention_mask = np.asarray(attention_mask, dtype=np.float32)
    Wq = np.asarray(Wq, dtype=np.float32)
    Wk = np.asarray(Wk, dtype=np.float32)
    Wv = np.asarray(Wv, dtype=np.float32)
    B, S, _ = hidden_states.shape
    nc = get_module(S)
    in_maps = [
        _core_inputs(hidden_states, attention_mask, Wq, Wk, Wv, c) for c in range(8)
    ]
    res = run_bass_kernel_spmd(nc, in_maps, core_ids=list(range(8)))
    out = np.empty((B, S, F), dtype=np.float32)
    for c in range(8):
        b, g = divmod(c, 4)
        out[b, :, g * HPC * D:(g + 1) * HPC * D] = res.results[c]["out"]
    return out
